# revision 46
# baseline (speedup 1.0000x reference)
"""Trainium2 Bass kernel for nn_BitModel (MLGRU step + BitGLU, ternary weights).

Strategy: data-parallel over the 4096 batch dim (512 rows per core, zero
collectives) + Strassen level-1 on the two big BitGLU matmuls.

Precision scheme (CPU-emulated rel err 1.707e-2 vs the 2e-2 gate; the
emulator matched the previous build's hw error to 2e-4): x runs f32r
(12-bit moving rounding; x-noise is amplified ~84x by the saturated-gate
transition bands, so it gets the most bits), gh runs f32r (cheap: phase 2
is only 256 matmuls), o and gu run fp16 against fp8 ternary stationaries.
All PSUM accumulation f32.

Strassen level 1 is applied to phase 3 (u/g2 = ACT(o @ W), k=2048 j=8192)
and phase 4 (y = gu @ Wy, k=8192 j=2048): 7 quadrant products replace 8,
cutting those phases' PE row-time by 12.5%. The weight-side combos
(sums/differences of ternary quadrants, values in {-2..2}; x16-scaled to
{-32..32} for phase 4 so the epilogue scale disappears) are precomputed on
host and stay exact in fp8e4m3. The moving-side combos are built by DVE in
fp16; emulation shows the extra rounding costs only ~4e-4 of rel err
because o/gu noise is weakly amplified. Each quadrant product is a
[128,256] PSUM half-bank accumulation; C-assembly is scalar_tensor_tensor
on DVE (bias fused into the final combine for phase 4).

On-device dataflow is feature-major throughout: [128 partitions =
feature % 128, feature_chunk, batch] with batch 512 (phases 1-2) or 256
halves (Strassen phases). No transposes anywhere on device.
"""

import sys

sys.path.insert(0, "/opt/trn_rl_repo")

import numpy as np

import concourse.bass as bass
import concourse.mybir as mybir
import concourse.tile as tile
from concourse.vector_clock import ScopedClock

DIM = 2048
HID = 8192
BATCH = 4096
NCORES = 8
B = BATCH // NCORES  # 512 batch rows per core
BH = B // 2  # 256: Strassen batch half
P = 128
JC_D = DIM // P  # 16 feature chunks for DIM
JC_H = HID // P  # 64 feature chunks for HID
JQ3 = 32  # phase-3 j-quadrant chunks (4096/128)
KK3 = 8   # phase-3 k-chunks per quadrant (1024/128)
JQ4 = 8   # phase-4 j-quadrant chunks (1024/128)
KK4 = 32  # phase-4 k-chunks per quadrant (4096/128)
THRESH = 0.33
GU_SCALE = 16.0  # gu is stored as gu/16 in fp16 to stay inside fp16 range

F16 = mybir.dt.float16
F32 = mybir.dt.float32
F32R = mybir.dt.float32r
F8 = mybir.dt.float8e4
ALU = mybir.AluOpType

# bias column layout in the packed [128, 208] bias tensor
COL_NF = 0  # -f_gate_b (negated: we compute 1-f = sigmoid(-(t+b)))
COL_C = 16
COL_G = 32
COL_O = 48
COL_U = 64
COL_G2 = 128
COL_Y = 192
N_BIAS_COLS = 208

# Strassen: M_i = (sum of A quadrants) @ (sum of B quadrants)
#   A [j, k] quadrants A(jh)(kh); B [k, b] quadrants B(kh)(bh)
# C11 = M1+M4-M5+M7; C12 = M3+M5; C21 = M2+M4; C22 = M1-M2+M3+M6
# M5 ships NEGATED (M5' = -M5) so C11 = M1+M4+M5'+M7 is all-additive:
# M6/M7 then accumulate straight onto the M1/M5' PSUM banks (start=False),
# saving DVE assembly ops: C11 = (M1+M4)+(M5'+M7), C12 = M3-M5'.
SA_TERMS = [
    [(1, 0, 0), (1, 1, 1)],    # M1 = A11 + A22
    [(1, 1, 0), (1, 1, 1)],    # M2 = A21 + A22
    [(1, 0, 0)],               # M3 = A11
    [(1, 1, 1)],               # M4 = A22
    [(-1, 0, 0), (-1, 0, 1)],  # M5' = -(A11 + A12)
    [(1, 1, 0), (-1, 0, 0)],   # M6 = A21 - A11
    [(1, 0, 1), (-1, 1, 1)],   # M7 = A12 - A22
]


def _patch_tile_drain():
    """This walrus build rejects instructions carrying >~2 attached sem
    waits ("Too many sync wait commands") and Tile's kernel-tail drain
    carries one wait per active logical proc. Re-emit those waits as
    standalone wait_ge instructions (1 wait each) before a wait-free
    drain."""
    if getattr(tile.TileContext, "_drain_patched", False):
        return

    def _drain_and_barrier(self, tick_clock, wait_clock):
        nc = self.nc
        probe = nc.sync.nop(nofuse=True)
        wait_clock.add_sem_waits(
            probe.ins, ScopedClock({None: tick_clock.global_clock})
        )
        si = probe.ins.sync_info
        waits = list(si.on_wait) if si else []
        if si:
            si.on_wait = []
        handles = {h.name: h for h in self.sems.allocated().values()}
        for w in waits:
            nc.sync.wait_ge(handles[w.ant_name], w.wait_value)
        nc.sync.drain()
        nc.all_engine_barrier()
        assert self.sems is not None
        popped = nc._tile_sem_poison_stack.pop()
        assert popped is self._sem_poison
        nc.clear_and_free_semaphores(list(self.sems.allocated().values()))
        nc.all_engine_barrier()

    tile.TileContext._drain_and_barrier = _drain_and_barrier
    tile.TileContext._drain_patched = True


_patch_tile_drain()


def _split_excess_waits(nc, cap=1, bundle=1):
    """This walrus build rejects instructions carrying more than ~2 attached
    sem waits ("Too many sync wait commands"). Tile attaches one wait per
    depended-on logical proc. Rewrite every instruction with >cap waits into
    a chain of single-wait InstEventSemaphore ops (what raw wait_ge emits,
    known-good) followed by the instruction keeping only `cap` waits."""
    ctr = 0
    for f in nc.m.functions:
        for bb in f.blocks:
            il = bb.instructions
            i = 0
            while i < len(il):
                inst = il[i]
                si = inst.sync_info
                waits = list(si.on_wait) if si else []
                if len(waits) > cap:
                    extra, keep = waits[:-cap], waits[-cap:]
                    evs = []
                    for j in range(0, len(extra), bundle):
                        ev = mybir.InstEventSemaphore(
                            name=f"waitsplit-{ctr}", ins=[], outs=[]
                        )
                        ctr += 1
                        ev.engine = inst.engine
                        ev.sync_info = mybir.SyncInfo(
                            on_wait=extra[j : j + bundle], on_update=[]
                        )
                        evs.append(ev)
                    si.on_wait = keep
                    il[i:i] = evs
                    i += len(evs)
                i += 1
    return ctr


def _ternary(w):
    w = np.asarray(w, np.float32)
    return np.where(np.abs(w) < THRESH, 0.0, np.sign(w)).astype(np.float32)


def _pack_mat(t, dtype):
    """[out_f, in_f] small-int f32 -> transposed, tiled [jc, p, ko, j]
    with element = t[jc*128+j, ko*128+p]."""
    of, inf_ = t.shape
    jc, ko = of // P, inf_ // P
    r = t.reshape(jc, P, ko, P)  # [jc, j, ko, p]
    r = np.ascontiguousarray(r.transpose(0, 3, 2, 1))  # [jc, p, ko, j]
    return r.astype(dtype)


def _pack_weight(w, dtype=np.float32):
    return _pack_mat(_ternary(w), dtype)


def _pack_strassen(w, dtype, scale=1.0, conjoined=False):
    """W [out,in] -> [7, jq, P, kk, P] fp8 Strassen A-side quadrant combos
    of the ternarized weight (x scale). Values stay in {-2..2}*scale,
    exact in fp8e4m3. conjoined=True reorders to [jq, P, 7, kk, P] so one
    jq-group's 7 slabs are a single contiguous 7KB-per-partition transfer
    (one split DMA per pass instead of 7: the SP engine's ~0.6us/dma_start
    otherwise saturates)."""
    t = _ternary(w) * scale
    of, inf_ = t.shape
    jh, kh = of // 2, inf_ // 2
    quad = lambda j, k: t[j * jh : (j + 1) * jh, k * kh : (k + 1) * kh]
    packs = []
    for terms in SA_TERMS:
        c = sum(s * quad(j, k) for s, j, k in terms)
        packs.append(_pack_mat(c, dtype))
    st = np.stack(packs)  # [7, jq, P, kk, P]
    if conjoined:
        st = np.ascontiguousarray(st.transpose(1, 2, 0, 3, 4))
    return st


def _pack_x(x_shard):
    """[B, DIM] f32 -> [p, ko, b] f32."""
    b, inf_ = x_shard.shape
    return np.ascontiguousarray(
        x_shard.reshape(b, inf_ // P, P).transpose(2, 1, 0)
    ).astype(np.float32)


def _pack_bias_col(b):
    """[out_f] -> [128, out_f//128] (partition-major)."""
    return np.ascontiguousarray(np.asarray(b, np.float32).reshape(-1, P).T)


def _build_nc():
    nc = bass.Bass()

    xT = nc.declare_dram_parameter("xT", [P, JC_D, B], F32R, isOutput=False)
    # gate weights ship as fp8 (ternary = exact) and are expanded to f32r
    # on-device by DVE: the f32r DMA would otherwise blow the HBM budget
    # during the x-stream window
    wf = nc.declare_dram_parameter("wf", [JC_D, P, JC_D, P], F8, isOutput=False)
    wc = nc.declare_dram_parameter("wc", [JC_D, P, JC_D, P], F8, isOutput=False)
    wg = nc.declare_dram_parameter("wg", [JC_D, P, JC_D, P], F8, isOutput=False)
    wo = nc.declare_dram_parameter("wo", [JC_D, P, JC_D, P], F8, isOutput=False)
    # Strassen combo weights: phase 3 conjoined [jq, P, 7, kk, P], phase 4
    # per-M [7, jq, P, kk, P]
    wuS = nc.declare_dram_parameter("wuS", [JQ3, P, 7, KK3, P], F8, isOutput=False)
    wg2S = nc.declare_dram_parameter("wg2S", [JQ3, P, 7, KK3, P], F8, isOutput=False)
    wo2S = nc.declare_dram_parameter("wo2S", [7, JQ4, P, KK4, P], F8, isOutput=False)
    biases = nc.declare_dram_parameter("biases", [P, N_BIAS_COLS], F32, isOutput=False)
    out = nc.declare_dram_parameter("out", [JC_D, P, B], F32, isOutput=True)

    AF = mybir.ActivationFunctionType
    from contextlib import ExitStack

    with tile.TileContext(nc) as tc:
        es_all = ExitStack()
        const = es_all.enter_context(tc.tile_pool(name="const", bufs=1))
        es_psumA = ExitStack()
        psum = es_psumA.enter_context(tc.tile_pool(name="psum", bufs=8, space="PSUM"))

        bias_sb = const.tile([P, N_BIAS_COLS], F32)

        # HAM clock-gate warmup: the PE defaults to 1.2GHz and unlocks
        # 2.4GHz only after one fully-busy free-running ~3.4us window;
        # a fully-idle window re-throttles. These dependency-free scratch
        # matmuls keep the PE busy through the DMA head window (first
        # real operands land ~12us in) so real work starts at full clock
        # with no >=3.4us gap in between.
        warm_w = const.tile([P, P], F16)
        warm_x = const.tile([P, B], F16)
        nc.vector.memset(warm_w[:], 1.0)
        nc.vector.memset(warm_x[:], 1.0)
        ps_warm = psum.tile([P, B], F32, tag="ps")
        for i in range(24):
            nc.tensor.matmul(
                ps_warm, warm_w[:], warm_x[:],
                start=(i == 0), stop=(i == 23), skip_group_check=True,
            )

        def bias_ap(col):
            return bias_sb[:, col : col + 1]

        def mm(ps, w_sb, act_sb, nk):
            for ko in range(nk):
                nc.tensor.matmul(
                    ps, w_sb[:, ko], act_sb[:, ko],
                    start=(ko == 0), stop=(ko == nk - 1),
                )

        # left-stack pools spanning phases 1-2
        es_p12 = ExitStack()
        gh_pool = es_p12.enter_context(tc.tile_pool(name="gh_pool", bufs=1))
        w8pool = es_p12.enter_context(tc.tile_pool(name="w8pool", bufs=8))
        wpool = es_p12.enter_context(tc.tile_pool(name="wpool", bufs=8))
        gh_sb = gh_pool.tile([P, JC_D, B], F32R)

        # right-stack pools whose lifetimes straddle the phase-2/3 boundary
        es_right = ExitStack()
        o_pool = es_right.enter_context(tc.tile_pool(name="o_pool", bufs=1, side="right"))
        o_sb = o_pool.tile([P, JC_D, B], F16)
        om_pool = es_right.enter_context(tc.tile_pool(name="om_pool", bufs=1, side="right"))
        # phase-3 moving operands (Strassen B-side of o): 5 fp16 combos;
        # the raw B11/B22 quadrants are views into o_sb
        OM = {m: om_pool.tile([P, KK3, BH], F16, name=f"om{m}") for m in (0, 2, 3, 5, 6)}

        def expand(src_dram, jc, stage=None):
            """DMA an fp8 ternary slab, DVE-expand it to f32r."""
            if stage is None:
                stage = w8pool.tile([P, JC_D, P], F8, tag="w8")
                nc.sync.dma_start(out=stage[:], in_=src_dram[jc])
            w_sb = wpool.tile([P, JC_D, P], F32R, tag="w512")
            nc.vector.tensor_copy(w_sb[:], stage[:])
            return w_sb

        # ---- phase 1: MLGRU gates; gh = g * ((1-f)*c) -> f32r ----
        with (
            tc.tile_pool(name="x_pool", bufs=1) as x_pool,
            tc.tile_pool(name="tmp1", bufs=2) as tmp,
        ):
            # DMA queues drain FIFO at aggregate ~300GB/s, so issue order
            # = landing order. Put the data that gates the first matmuls
            # (x chunk 0, the first gate slab, biases) ahead of the
            # 3.5MB x bulk so compute starts early.
            x_sb = x_pool.tile([P, JC_D, B], F32R)
            wf0_8 = w8pool.tile([P, JC_D, P], F8, tag="w8")
            nc.sync.dma_start(out=x_sb[:, 0:1], in_=xT[:, 0:1])
            nc.sync.dma_start(out=wf0_8[:], in_=wf[0])
            nc.sync.dma_start(out=bias_sb[:], in_=biases[:])
            nc.sync.dma_start(out=x_sb[:, 1:2], in_=xT[:, 1:2])
            XCH = 2
            for kc in range(1, JC_D // XCH):
                ks = slice(kc * XCH, (kc + 1) * XCH)
                nc.sync.dma_start(out=x_sb[:, ks], in_=xT[:, ks])

            def gate_epilogue(jc, ps_f, ps_c, ps_g):
                # 1-f = sigmoid(-(t+b)); bias column holds -b_f
                # sigmoid ops adjacent, silu last: fewer ACT table reloads
                onemf = tmp.tile([P, B], F32, tag="onemf")
                nc.scalar.activation(
                    onemf, ps_f, AF.Sigmoid, bias=bias_ap(COL_NF + jc), scale=-1.0
                )
                g_sb = tmp.tile([P, B], F32, tag="g")
                nc.scalar.activation(g_sb, ps_g, AF.Sigmoid, bias=bias_ap(COL_G + jc))
                c_sb = tmp.tile([P, B], F32, tag="c")
                nc.scalar.activation(c_sb, ps_c, AF.Silu, bias=bias_ap(COL_C + jc))
                h_sb = tmp.tile([P, B], F32, tag="h")
                nc.vector.tensor_mul(h_sb, onemf, c_sb)
                nc.vector.tensor_mul(gh_sb[:, jc], g_sb, h_sb)

            # The head runs ko-major across 7 open PSUM groups (jc 0-1 all
            # gates + jc 2's f gate; warmup bank + 7 = all 8 banks): each
            # arriving x chunk unlocks ~1.58us of matmuls, above its
            # arrival cadence, so the PE rides the x-transfer front
            # gap-free (recurring idle gaps re-throttle the HAM clock).
            GATES = (("f", wf), ("c", wc), ("g", wg))
            HEAD = [(0, "f"), (0, "c"), (0, "g"),
                    (1, "f"), (1, "c"), (1, "g"), (2, "f")]
            SRC = dict(GATES)
            stages, hw, hp = {}, {}, {}
            for jc, nm in HEAD:
                src = SRC[nm]
                if jc == 0 and nm == "f":
                    stages[jc, nm] = wf0_8
                else:
                    st = w8pool.tile(
                        [P, JC_D, P], F8, tag="w8", name=f"hs_{jc}{nm}"
                    )
                    nc.sync.dma_start(out=st[:], in_=src[jc])
                    stages[jc, nm] = st
                hw[jc, nm] = wpool.tile(
                    [P, JC_D, P], F32R, tag="w512", name=f"hw_{jc}{nm}"
                )
                hp[jc, nm] = psum.tile(
                    [P, B], F32, tag="ps", name=f"hp_{jc}{nm}"
                )
            # half-slab expands, all first halves before second halves,
            # so every slab's ko<8 columns are ready early
            HK = JC_D // 2
            for half in (slice(0, HK), slice(HK, JC_D)):
                for jc, nm in HEAD:
                    nc.vector.tensor_copy(
                        hw[jc, nm][:, half], stages[jc, nm][:, half]
                    )
            for ko in range(JC_D):
                for jc, nm in HEAD:
                    nc.tensor.matmul(
                        hp[jc, nm], hw[jc, nm][:, ko], x_sb[:, ko],
                        start=(ko == 0), stop=(ko == JC_D - 1),
                    )
            for jc in (0, 1):
                gate_epilogue(jc, hp[jc, "f"], hp[jc, "c"], hp[jc, "g"])

            for jc in range(2, JC_D):
                if jc == 2:
                    ps_f = hp[2, "f"]
                else:
                    wf_sb = expand(wf, jc)
                    ps_f = psum.tile([P, B], F32, tag="ps")
                    mm(ps_f, wf_sb, x_sb, JC_D)

                wc_sb = expand(wc, jc)
                ps_c = psum.tile([P, B], F32, tag="ps")
                mm(ps_c, wc_sb, x_sb, JC_D)

                wg_sb = expand(wg, jc)
                ps_g = psum.tile([P, B], F32, tag="ps")
                mm(ps_g, wg_sb, x_sb, JC_D)

                gate_epilogue(jc, ps_f, ps_c, ps_g)

        # ---- phase 3/4 shared defs (needed for prefetch from phase 2) ----
        # pass list: (weight set, jq), ACT-table-friendly order
        passes = []
        for jq in range(JQ3):
            pair = [("u", jq), ("g", jq)]
            if jq % 2:
                pair.reverse()
            passes += pair
        W3SRC = {"u": wuS, "g": wg2S}

        def issue_p3A(pi):
            """First slab piece (M1..M4, consumed from pass start)."""
            kind, jq = passes[pi]
            src = W3SRC[kind]
            t = w3pool.tile([P, 4, KK3, P], F8, tag="w3a")
            for s in range(4):
                pp = slice(s * (P // 4), (s + 1) * (P // 4))
                nc.sync.dma_start(out=t[pp], in_=src[jq, pp, 0:4])
            return t

        def issue_p3B(pi):
            """Second slab piece (M5..M7, consumed ~mid-pass)."""
            kind, jq = passes[pi]
            src = W3SRC[kind]
            t = w3pool.tile([P, 3, KK3, P], F8, tag="w3b")
            for s in range(4):
                pp = slice(s * (P // 4), (s + 1) * (P // 4))
                nc.sync.dma_start(out=t[pp], in_=src[jq, pp, 4:7])
            return t

        def issue_p4_slab(jq, m):
            # 8-way split (halves the in-flight latency vs the 3.4us/M
            # consumption rate); half issued from the Activation engine
            # (also hwdge-capable, idle in phase 4) to keep SP under ~60%
            t = w4pool.tile([P, KK4, P], F8, tag="w4")
            for s in range(8):
                pp = slice(s * (P // 8), (s + 1) * (P // 8))
                eng = nc.sync if s % 2 == 0 else nc.scalar
                eng.dma_start(out=t[pp], in_=wo2S[m, jq, pp])
            return t

        slabA = {}
        slabB = {}
        w4_prime = []

        # ---- phase 2: o = out_proj(gh) + b -> fp16; build phase-3 combos ----
        def om_combos(kk):
            """o chunks kk (k-half 1) and kk+8 (k-half 2) are both ready:
            emit the 5 fp16 Strassen combos for phase 3 at column kk."""
            o11 = o_sb[:, kk, 0:BH]
            o12 = o_sb[:, kk, BH:B]
            o21 = o_sb[:, kk + KK3, 0:BH]
            o22 = o_sb[:, kk + KK3, BH:B]
            nc.vector.scalar_tensor_tensor(OM[0][:, kk], o11, 1.0, o22, ALU.mult, ALU.add)
            nc.vector.scalar_tensor_tensor(OM[2][:, kk], o12, 1.0, o22, ALU.mult, ALU.subtract)
            nc.vector.scalar_tensor_tensor(OM[3][:, kk], o21, 1.0, o11, ALU.mult, ALU.subtract)
            nc.vector.scalar_tensor_tensor(OM[5][:, kk], o11, 1.0, o12, ALU.mult, ALU.add)
            nc.vector.scalar_tensor_tensor(OM[6][:, kk], o21, 1.0, o22, ALU.mult, ALU.add)

        # w3pool opens at phase-2 start (x_pool/tmp1 are gone, so it fits)
        # and passes 0/1 are primed HERE: SP executes dma_starts in program
        # order and stalls on w8pool WAR waits, so it only reaches these
        # after phase 1 — they land during phase 2, not at the boundary.
        w3pool = es_right.enter_context(tc.tile_pool(name="w3pool", bufs=3, side="right"))
        slabA[0] = issue_p3A(0)
        slabB[0] = issue_p3B(0)
        slabA[1] = issue_p3A(1)
        slabB[1] = issue_p3B(1)

        for jc in range(JC_D):
            wo_sb = expand(wo, jc)
            ps_o = psum.tile([P, B], F32, tag="ps")
            mm(ps_o, wo_sb, gh_sb, JC_D)
            nc.vector.tensor_scalar_add(o_sb[:, jc], ps_o, bias_ap(COL_O + jc))
            if jc >= KK3:
                om_combos(jc - KK3)

        es_p12.close()  # frees wpool, w8pool, gh (also x_pool/tmp1 already closed)
        es_psumA.close()

        # ---- phases 3-4: Strassen level-1 machinery ----
        # (pools open only now: SBUF space is reserved at pool-open, and these
        # only fit once the phase-1/2 pools are gone)
        es_psumB = ExitStack()
        psumS = es_psumB.enter_context(tc.tile_pool(name="psumS", bufs=8, space="PSUM"))
        pb_pool = es_right.enter_context(tc.tile_pool(name="pb_pool", bufs=1, side="right"))
        # phase-4 moving operands (Strassen B-side of gu): 7 fp16 tensors
        # (indices 1 and 4 are the raw B11/B22 quadrants, written directly)
        PB = [pb_pool.tile([P, KK4, BH], F16, name=f"pb{m}") for m in range(7)]
        w4pool = es_right.enter_context(tc.tile_pool(name="w4pool", bufs=4, side="right"))
        s_tmp = es_right.enter_context(tc.tile_pool(name="s_tmp", bufs=12, side="right"))

        def strassen_group(kk_n, slab, mover, combine):
            """Emit the 7 quadrant products and C-assembly for one j-group.
            slabs[m]: SBUF fp8 stationary [P, kk_n, P]; mover(m, kk): fp16
            moving [P, BH] AP; combine(quad, in0_sbuf, in1_psum, op1, eng)
            emits the final combining op for C_quad. M1/M2/M3 get SBUF
            copies (a DVE op may read at most one PSUM source); M6/M7
            accumulate onto the M1/M5' banks. Assembly ops are split
            between DVE ("v") and the otherwise-idle GpSimd ("g") so
            neither queue backlogs the PSUM-bank recycling the PE needs."""
            def run_m(m, ps=None):
                start = ps is None
                if start:
                    ps = psumS.tile([P, BH], F32, tag="psS")
                for kk in range(kk_n):
                    nc.tensor.matmul(
                        ps, slab(m, kk), mover(m, kk),
                        start=(start and kk == 0), stop=(kk == kk_n - 1),
                        skip_group_check=not start,
                    )
                return ps

            def to_sbuf(ps, scalar=False):
                t = s_tmp.tile([P, BH], F32, tag="t")
                if scalar:
                    # ScalarE copy: 'copy' is in every ACT table (no reload),
                    # and this keeps the PSUM-freeing path off the DVE queue
                    nc.scalar.activation(t, ps, AF.Copy)
                else:
                    nc.vector.tensor_copy(t, ps)
                return t

            mA = run_m(0)           # bank A = M1
            m1s = to_sbuf(mA, scalar=True)
            mB = run_m(1)           # bank B = M2
            m2s = to_sbuf(mB, scalar=True)
            mC = run_m(3)           # bank C = M4
            c11a = s_tmp.tile([P, BH], F32, tag="t")
            nc.vector.scalar_tensor_tensor(c11a, m1s, 1.0, mC, ALU.mult, ALU.add)
            combine(21, m2s, mC, ALU.add, "v")       # C21 = M2 + M4
            mD = run_m(2)           # bank D = M3
            m3s = to_sbuf(mD)
            mE = run_m(4)           # bank E = M5' = -M5
            combine(12, m3s, mE, ALU.subtract, "v")  # C12 = M3 - M5'
            t22 = s_tmp.tile([P, BH], F32, tag="t")
            nc.vector.scalar_tensor_tensor(t22, m3s, 1.0, m2s, ALU.mult, ALU.subtract)
            run_m(5, ps=mA)         # bank A = M1 + M6
            combine(22, t22, mA, ALU.add, "v")       # C22 = (M3-M2)+(M1+M6)
            run_m(6, ps=mE)         # bank E = M5' + M7
            combine(11, c11a, mE, ALU.add, "v")      # C11 = (M1+M4)+(M5'+M7)

        # ---- phase 3: BitGLU u/g2 via Strassen; gu -> phase-4 operands ----
        # movers: combos + raw o-quadrant views (B11 = o[kh1, bh1],
        # B22 = o[kh2, bh2])
        def mover3(m, kk):
            if m == 1:
                return o_sb[:, kk, 0:BH]
            if m == 4:
                return o_sb[:, KK3 + kk, BH:B]
            return OM[m][:, kk]

        with (
            tc.tile_pool(name="stash", bufs=5, side="right") as stash_pool,
            tc.tile_pool(name="p3b", bufs=4, side="right") as p3b,
        ):
            stash = {}
            bt = {}
            for pi, (kind, jq) in enumerate(passes):
                # prefetch: early piece 2 passes ahead, late piece 1 ahead
                if pi + 2 < len(passes) and pi + 2 not in slabA:
                    slabA[pi + 2] = issue_p3A(pi + 2)
                if pi + 1 < len(passes) and pi + 1 not in slabB:
                    slabB[pi + 1] = issue_p3B(pi + 1)
                tA = slabA.pop(pi)
                tB = slabB.pop(pi)
                first = (pi % 2) == 0
                func = AF.Silu if kind == "u" else AF.Sigmoid
                colb = COL_U if kind == "u" else COL_G2

                def combine(quad, a, b_, op1, eng, _jq=jq, _first=first, _func=func, _colb=colb):
                    c = s_tmp.tile([P, BH], F32, tag="t")
                    e = nc.vector if eng == "v" else nc.gpsimd
                    e.scalar_tensor_tensor(c, a, 1.0, b_, ALU.mult, op1)
                    jout = _jq if quad in (11, 12) else _jq + JQ3
                    if _first:
                        dst = stash_pool.tile([P, BH], F32, tag="s1")
                        nc.scalar.activation(dst, c, _func, bias=bias_ap(_colb + jout))
                        stash[quad] = dst
                    else:
                        act = s_tmp.tile([P, BH], F32, tag="t")
                        nc.scalar.activation(act, c, _func, bias=bias_ap(_colb + jout))
                        # gu quadrant = (act/16) * stashed other factor
                        if quad == 11:
                            dst = PB[1][:, _jq]
                        elif quad == 22:
                            dst = PB[4][:, _jq]
                        else:
                            dst = p3b.tile([P, BH], F16, tag="bt", name=f"bt{quad}")
                            bt[quad] = dst
                        nc.vector.scalar_tensor_tensor(
                            dst, act, 1.0 / GU_SCALE, stash[quad], ALU.mult, ALU.mult
                        )
                        if quad == 11:
                            # last quad of the pair: emit phase-4 combos at
                            # kk=jq (GpSimd: keep them off the DVE queue)
                            b11 = PB[1][:, _jq]
                            b22 = PB[4][:, _jq]
                            b12 = bt[12]
                            b21 = bt[21]
                            nc.vector.scalar_tensor_tensor(PB[0][:, _jq], b11, 1.0, b22, ALU.mult, ALU.add)
                            nc.vector.scalar_tensor_tensor(PB[2][:, _jq], b12, 1.0, b22, ALU.mult, ALU.subtract)
                            nc.vector.scalar_tensor_tensor(PB[3][:, _jq], b21, 1.0, b11, ALU.mult, ALU.subtract)
                            nc.vector.scalar_tensor_tensor(PB[5][:, _jq], b11, 1.0, b12, ALU.mult, ALU.add)
                            nc.vector.scalar_tensor_tensor(PB[6][:, _jq], b21, 1.0, b22, ALU.mult, ALU.add)

                strassen_group(
                    KK3,
                    lambda m, kk, _a=tA, _b=tB: (
                        _a[:, m, kk] if m < 4 else _b[:, m - 4, kk]
                    ),
                    mover3, combine,
                )
                if pi == len(passes) - 2:
                    # prime phase-4 slab prefetch (first two M's of group 0)
                    w4_prime.append(issue_p4_slab(0, 0))
                    w4_prime.append(issue_p4_slab(0, 1))

        with tc.tile_pool(name="outp", bufs=4, side="right") as outp:
            pending = {}
            for jq in range(JQ4):
                for m in range(7):
                    if jq == 0 and m < 2:
                        pending[(jq, m)] = w4_prime[m]
                    g2, m2_ = (jq, m + 2) if m + 2 < 7 else (jq + 1, m + 2 - 7)
                    if g2 < JQ4 and (g2, m2_) not in pending:
                        pending[(g2, m2_)] = issue_p4_slab(g2, m2_)
                slabs = [pending.pop((jq, m)) for m in range(7)]

                def combine4(quad, a, b_, op1, eng, _jq=jq):
                    jout = _jq if quad in (11, 12) else _jq + JQ4
                    boff = 0 if quad in (11, 21) else BH
                    y = outp.tile([P, BH], F32, tag="y")
                    e = nc.vector if eng == "v" else nc.gpsimd
                    e.scalar_tensor_tensor(
                        y, a, bias_ap(COL_Y + jout), b_, ALU.add, op1
                    )
                    ns = 4 if _jq == JQ4 - 1 else 2
                    for s in range(ns):
                        pp = slice(s * (P // ns), (s + 1) * (P // ns))
                        nc.sync.dma_start(
                            out=out[jout, pp, boff : boff + BH], in_=y[pp]
                        )

                strassen_group(
                    KK4, lambda m, kk, _s=slabs: _s[m][:, kk],
                    lambda m, kk: PB[m][:, kk], combine4,
                )

        es_right.close()
        es_psumB.close()
        es_all.close()

    _split_excess_waits(nc)
    return nc


def prep_in_maps(inputs):
    """Build the 8 per-core input maps from the full-size inputs."""
    import ml_dtypes

    E4 = ml_dtypes.float8_e4m3
    x = np.asarray(inputs["x"], np.float32)

    shared = {
        "wf": _pack_weight(inputs["f_gate_w"], dtype=E4),
        "wc": _pack_weight(inputs["c_proj_w"], dtype=E4),
        "wg": _pack_weight(inputs["g_gate_w"], dtype=E4),
        "wo": _pack_weight(inputs["out_proj_w"], dtype=E4),
        "wuS": _pack_strassen(inputs["proj_u_w"], E4, conjoined=True),
        "wg2S": _pack_strassen(inputs["proj_g_w"], E4, conjoined=True),
        # x16: undoes the gu/16 fp16 storage scale so no epilogue multiply
        "wo2S": _pack_strassen(inputs["proj_out_w"], E4, scale=GU_SCALE),
    }
    bias = np.zeros((P, N_BIAS_COLS), np.float32)
    bias[:, COL_NF:COL_NF + JC_D] = _pack_bias_col(-np.asarray(inputs["f_gate_b"]))
    bias[:, COL_C:COL_C + JC_D] = _pack_bias_col(inputs["c_proj_b"])
    bias[:, COL_G:COL_G + JC_D] = _pack_bias_col(inputs["g_gate_b"])
    bias[:, COL_O:COL_O + JC_D] = _pack_bias_col(inputs["out_proj_b"])
    bias[:, COL_U:COL_U + JC_H] = _pack_bias_col(inputs["proj_u_b"])
    bias[:, COL_G2:COL_G2 + JC_H] = _pack_bias_col(inputs["proj_g_b"])
    bias[:, COL_Y:COL_Y + JC_D] = _pack_bias_col(inputs["proj_out_b"])
    shared["biases"] = bias

    in_maps = []
    for core in range(NCORES):
        m = dict(shared)
        m["xT"] = _pack_x(x[core * B : (core + 1) * B])
        in_maps.append(m)
    return in_maps


def gather_output(results):
    """results[i]['out'] is [JC_D, P, B]; assemble full [BATCH, DIM] f32."""
    parts = []
    for core in range(NCORES):
        y = np.asarray(results[core]["out"], np.float32)  # [jc, p, b]
        parts.append(y.reshape(DIM, B).T)  # [b, j]
    return np.ascontiguousarray(np.concatenate(parts, axis=0))


_NC_CACHE = []


def run(inputs, trace=False, **kw):
    from concourse.bass_utils import run_bass_kernel_spmd

    if not _NC_CACHE:
        _NC_CACHE.append(_build_nc())
    nc = _NC_CACHE[0]
    in_maps = prep_in_maps(inputs)
    res = run_bass_kernel_spmd(nc, in_maps, core_ids=list(range(NCORES)), trace=trace, **kw)
    return res


def kernel(**inputs):
    res = run(inputs, trace=False)
    return gather_output(res.results)


# revision 48
# speedup vs baseline: 1.0880x; 1.0880x over previous
"""Trainium2 Bass kernel for nn_BitModel (MLGRU step + BitGLU, ternary weights).

Strategy: data-parallel over the 4096 batch dim (512 rows per core, zero
collectives) + Strassen level-1 on the two big BitGLU matmuls.

Precision scheme (CPU-emulated rel err 1.707e-2 vs the 2e-2 gate; the
emulator matched the previous build's hw error to 2e-4): x runs f32r
(12-bit moving rounding; x-noise is amplified ~84x by the saturated-gate
transition bands, so it gets the most bits), gh runs f32r (cheap: phase 2
is only 256 matmuls), o and gu run fp16 against fp8 ternary stationaries.
All PSUM accumulation f32.

Strassen level 1 is applied to phase 3 (u/g2 = ACT(o @ W), k=2048 j=8192)
and phase 4 (y = gu @ Wy, k=8192 j=2048): 7 quadrant products replace 8,
cutting those phases' PE row-time by 12.5%. The weight-side combos
(sums/differences of ternary quadrants, values in {-2..2}; x16-scaled to
{-32..32} for phase 4 so the epilogue scale disappears) are precomputed on
host and stay exact in fp8e4m3. The moving-side combos are built by DVE in
fp16; emulation shows the extra rounding costs only ~4e-4 of rel err
because o/gu noise is weakly amplified. Each quadrant product is a
[128,256] PSUM half-bank accumulation; C-assembly is scalar_tensor_tensor
on DVE (bias fused into the final combine for phase 4).

On-device dataflow is feature-major throughout: [128 partitions =
feature % 128, feature_chunk, batch] with batch 512 (phases 1-2) or 256
halves (Strassen phases). No transposes anywhere on device.
"""

import sys

sys.path.insert(0, "/opt/trn_rl_repo")

import numpy as np

import concourse.bass as bass
import concourse.mybir as mybir
import concourse.tile as tile
from concourse.vector_clock import ScopedClock

DIM = 2048
HID = 8192
BATCH = 4096
NCORES = 8
B = BATCH // NCORES  # 512 batch rows per core
BH = B // 2  # 256: Strassen batch half
P = 128
JC_D = DIM // P  # 16 feature chunks for DIM
JC_H = HID // P  # 64 feature chunks for HID
JQ3 = 32  # phase-3 j-quadrant chunks (4096/128)
KK3 = 8   # phase-3 k-chunks per quadrant (1024/128)
JQ4 = 8   # phase-4 j-quadrant chunks (1024/128)
KK4 = 32  # phase-4 k-chunks per quadrant (4096/128)
THRESH = 0.33
GU_SCALE = 16.0  # gu is stored as gu/16 in fp16 to stay inside fp16 range

F16 = mybir.dt.float16
F32 = mybir.dt.float32
F32R = mybir.dt.float32r
F8 = mybir.dt.float8e4
ALU = mybir.AluOpType

# bias column layout in the packed [128, 208] bias tensor
COL_NF = 0  # -f_gate_b (negated: we compute 1-f = sigmoid(-(t+b)))
COL_C = 16
COL_G = 32
COL_O = 48
COL_U = 64
COL_G2 = 128
COL_Y = 192
N_BIAS_COLS = 208

# Strassen: M_i = (sum of A quadrants) @ (sum of B quadrants)
#   A [j, k] quadrants A(jh)(kh); B [k, b] quadrants B(kh)(bh)
# C11 = M1+M4-M5+M7; C12 = M3+M5; C21 = M2+M4; C22 = M1-M2+M3+M6
# M5 ships NEGATED (M5' = -M5) so C11 = M1+M4+M5'+M7 is all-additive:
# M6/M7 then accumulate straight onto the M1/M5' PSUM banks (start=False),
# saving DVE assembly ops: C11 = (M1+M4)+(M5'+M7), C12 = M3-M5'.
SA_TERMS = [
    [(1, 0, 0), (1, 1, 1)],    # M1 = A11 + A22
    [(1, 1, 0), (1, 1, 1)],    # M2 = A21 + A22
    [(1, 0, 0)],               # M3 = A11
    [(1, 1, 1)],               # M4 = A22
    [(-1, 0, 0), (-1, 0, 1)],  # M5' = -(A11 + A12)
    [(1, 1, 0), (-1, 0, 0)],   # M6 = A21 - A11
    [(1, 0, 1), (-1, 1, 1)],   # M7 = A12 - A22
]


def _patch_tile_drain():
    """This walrus build rejects instructions carrying >~2 attached sem
    waits ("Too many sync wait commands") and Tile's kernel-tail drain
    carries one wait per active logical proc. Re-emit those waits as
    standalone wait_ge instructions (1 wait each) before a wait-free
    drain."""
    if getattr(tile.TileContext, "_drain_patched", False):
        return

    def _drain_and_barrier(self, tick_clock, wait_clock):
        nc = self.nc
        probe = nc.sync.nop(nofuse=True)
        wait_clock.add_sem_waits(
            probe.ins, ScopedClock({None: tick_clock.global_clock})
        )
        si = probe.ins.sync_info
        waits = list(si.on_wait) if si else []
        if si:
            si.on_wait = []
        handles = {h.name: h for h in self.sems.allocated().values()}
        for w in waits:
            nc.sync.wait_ge(handles[w.ant_name], w.wait_value)
        nc.sync.drain()
        nc.all_engine_barrier()
        assert self.sems is not None
        popped = nc._tile_sem_poison_stack.pop()
        assert popped is self._sem_poison
        nc.clear_and_free_semaphores(list(self.sems.allocated().values()))
        nc.all_engine_barrier()

    tile.TileContext._drain_and_barrier = _drain_and_barrier
    tile.TileContext._drain_patched = True


_patch_tile_drain()


def _split_excess_waits(nc, cap=1, bundle=1):
    """This walrus build rejects instructions carrying more than ~2 attached
    sem waits ("Too many sync wait commands"). Tile attaches one wait per
    depended-on logical proc. Rewrite every instruction with >cap waits into
    a chain of single-wait InstEventSemaphore ops (what raw wait_ge emits,
    known-good) followed by the instruction keeping only `cap` waits."""
    ctr = 0
    for f in nc.m.functions:
        for bb in f.blocks:
            il = bb.instructions
            i = 0
            while i < len(il):
                inst = il[i]
                si = inst.sync_info
                waits = list(si.on_wait) if si else []
                if len(waits) > cap:
                    extra, keep = waits[:-cap], waits[-cap:]
                    evs = []
                    for j in range(0, len(extra), bundle):
                        ev = mybir.InstEventSemaphore(
                            name=f"waitsplit-{ctr}", ins=[], outs=[]
                        )
                        ctr += 1
                        ev.engine = inst.engine
                        ev.sync_info = mybir.SyncInfo(
                            on_wait=extra[j : j + bundle], on_update=[]
                        )
                        evs.append(ev)
                    si.on_wait = keep
                    il[i:i] = evs
                    i += len(evs)
                i += 1
    return ctr


def _ternary(w):
    w = np.asarray(w, np.float32)
    return np.where(np.abs(w) < THRESH, 0.0, np.sign(w)).astype(np.float32)


def _pack_mat(t, dtype):
    """[out_f, in_f] small-int f32 -> transposed, tiled [jc, p, ko, j]
    with element = t[jc*128+j, ko*128+p]."""
    of, inf_ = t.shape
    jc, ko = of // P, inf_ // P
    r = t.reshape(jc, P, ko, P)  # [jc, j, ko, p]
    r = np.ascontiguousarray(r.transpose(0, 3, 2, 1))  # [jc, p, ko, j]
    return r.astype(dtype)


def _pack_weight(w, dtype=np.float32):
    return _pack_mat(_ternary(w), dtype)


def _pack_strassen(w, dtype, scale=1.0, conjoined=False):
    """W [out,in] -> [7, jq, P, kk, P] fp8 Strassen A-side quadrant combos
    of the ternarized weight (x scale). Values stay in {-2..2}*scale,
    exact in fp8e4m3. conjoined=True reorders to [jq, P, 7, kk, P] so one
    jq-group's 7 slabs are a single contiguous 7KB-per-partition transfer
    (one split DMA per pass instead of 7: the SP engine's ~0.6us/dma_start
    otherwise saturates)."""
    t = _ternary(w) * scale
    of, inf_ = t.shape
    jh, kh = of // 2, inf_ // 2
    quad = lambda j, k: t[j * jh : (j + 1) * jh, k * kh : (k + 1) * kh]
    packs = []
    for terms in SA_TERMS:
        c = sum(s * quad(j, k) for s, j, k in terms)
        packs.append(_pack_mat(c, dtype))
    st = np.stack(packs)  # [7, jq, P, kk, P]
    if conjoined:
        st = np.ascontiguousarray(st.transpose(1, 2, 0, 3, 4))
    return st


def _pack_x(x_shard):
    """[B, DIM] f32 -> [p, ko, b] f32."""
    b, inf_ = x_shard.shape
    return np.ascontiguousarray(
        x_shard.reshape(b, inf_ // P, P).transpose(2, 1, 0)
    ).astype(np.float32)


def _pack_bias_col(b):
    """[out_f] -> [128, out_f//128] (partition-major)."""
    return np.ascontiguousarray(np.asarray(b, np.float32).reshape(-1, P).T)


def _build_nc():
    nc = bass.Bass()

    xT = nc.declare_dram_parameter("xT", [P, JC_D, B], F32R, isOutput=False)
    # gate weights ship as fp8 (ternary = exact) and are expanded to f32r
    # on-device by DVE: the f32r DMA would otherwise blow the HBM budget
    # during the x-stream window
    wf = nc.declare_dram_parameter("wf", [JC_D, P, JC_D, P], F8, isOutput=False)
    wc = nc.declare_dram_parameter("wc", [JC_D, P, JC_D, P], F8, isOutput=False)
    wg = nc.declare_dram_parameter("wg", [JC_D, P, JC_D, P], F8, isOutput=False)
    wo = nc.declare_dram_parameter("wo", [JC_D, P, JC_D, P], F8, isOutput=False)
    # Strassen combo weights: phase 3 conjoined [jq, P, 7, kk, P], phase 4
    # per-M [7, jq, P, kk, P]
    wuS = nc.declare_dram_parameter("wuS", [JQ3, P, 7, KK3, P], F8, isOutput=False)
    wg2S = nc.declare_dram_parameter("wg2S", [JQ3, P, 7, KK3, P], F8, isOutput=False)
    wo2S = nc.declare_dram_parameter("wo2S", [7, JQ4, P, KK4, P], F8, isOutput=False)
    biases = nc.declare_dram_parameter("biases", [P, N_BIAS_COLS], F32, isOutput=False)
    out = nc.declare_dram_parameter("out", [JC_D, P, B], F32, isOutput=True)

    AF = mybir.ActivationFunctionType
    from contextlib import ExitStack

    with tile.TileContext(nc) as tc:
        es_all = ExitStack()
        const = es_all.enter_context(tc.tile_pool(name="const", bufs=1))
        es_psumA = ExitStack()
        psum = es_psumA.enter_context(tc.tile_pool(name="psum", bufs=8, space="PSUM"))

        bias_sb = const.tile([P, N_BIAS_COLS], F32)

        # HAM clock-gate warmup: the PE defaults to 1.2GHz and unlocks
        # 2.4GHz only after one fully-busy free-running ~3.4us window;
        # a fully-idle window re-throttles. These dependency-free scratch
        # matmuls keep the PE busy through the DMA head window (first
        # real operands land ~12us in) so real work starts at full clock
        # with no >=3.4us gap in between.
        warm_w = const.tile([P, P], F16)
        warm_x = const.tile([P, B], F16)
        nc.vector.memset(warm_w[:], 1.0)
        nc.vector.memset(warm_x[:], 1.0)
        ps_warm = psum.tile([P, B], F32, tag="ps")
        for i in range(24):
            nc.tensor.matmul(
                ps_warm, warm_w[:], warm_x[:],
                start=(i == 0), stop=(i == 23), skip_group_check=True,
            )

        def bias_ap(col):
            return bias_sb[:, col : col + 1]

        def mm(ps, w_sb, act_sb, nk):
            for ko in range(nk):
                nc.tensor.matmul(
                    ps, w_sb[:, ko], act_sb[:, ko],
                    start=(ko == 0), stop=(ko == nk - 1),
                )

        # left-stack pools spanning phases 1-2
        es_p12 = ExitStack()
        gh_pool = es_p12.enter_context(tc.tile_pool(name="gh_pool", bufs=1))
        w8pool = es_p12.enter_context(tc.tile_pool(name="w8pool", bufs=8))
        wpool = es_p12.enter_context(tc.tile_pool(name="wpool", bufs=8))
        gh_sb = gh_pool.tile([P, JC_D, B], F32R)

        # right-stack pools whose lifetimes straddle the phase-2/3 boundary
        es_right = ExitStack()
        o_pool = es_right.enter_context(tc.tile_pool(name="o_pool", bufs=1, side="right"))
        o_sb = o_pool.tile([P, JC_D, B], F16)
        om_pool = es_right.enter_context(tc.tile_pool(name="om_pool", bufs=1, side="right"))
        # phase-3 moving operands (Strassen B-side of o): 5 fp16 combos;
        # the raw B11/B22 quadrants are views into o_sb
        OM = {m: om_pool.tile([P, KK3, BH], F16, name=f"om{m}") for m in (0, 2, 3, 5, 6)}

        def expand(src_dram, jc, stage=None):
            """DMA an fp8 ternary slab, DVE-expand it to f32r."""
            if stage is None:
                stage = w8pool.tile([P, JC_D, P], F8, tag="w8")
                nc.sync.dma_start(out=stage[:], in_=src_dram[jc])
            w_sb = wpool.tile([P, JC_D, P], F32R, tag="w512")
            nc.vector.tensor_copy(w_sb[:], stage[:])
            return w_sb

        # ---- phase 1: MLGRU gates; gh = g * ((1-f)*c) -> f32r ----
        with (
            tc.tile_pool(name="x_pool", bufs=1) as x_pool,
            tc.tile_pool(name="tmp1", bufs=2) as tmp,
        ):
            # DMA queues drain FIFO at aggregate ~300GB/s, so issue order
            # = landing order. Put the data that gates the first matmuls
            # (x chunk 0, the first gate slab, biases) ahead of the
            # 3.5MB x bulk so compute starts early.
            x_sb = x_pool.tile([P, JC_D, B], F32R)
            wf0_8 = w8pool.tile([P, JC_D, P], F8, tag="w8")
            nc.sync.dma_start(out=x_sb[:, 0:1], in_=xT[:, 0:1])
            nc.sync.dma_start(out=wf0_8[:], in_=wf[0])
            nc.sync.dma_start(out=bias_sb[:], in_=biases[:])
            nc.sync.dma_start(out=x_sb[:, 1:2], in_=xT[:, 1:2])
            XCH = 2
            for kc in range(1, JC_D // XCH):
                ks = slice(kc * XCH, (kc + 1) * XCH)
                nc.sync.dma_start(out=x_sb[:, ks], in_=xT[:, ks])

            def gate_epilogue(jc, ps_f, ps_c, ps_g):
                # 1-f = sigmoid(-(t+b)); bias column holds -b_f
                # sigmoid ops adjacent, silu last: fewer ACT table reloads
                onemf = tmp.tile([P, B], F32, tag="onemf")
                nc.scalar.activation(
                    onemf, ps_f, AF.Sigmoid, bias=bias_ap(COL_NF + jc), scale=-1.0
                )
                g_sb = tmp.tile([P, B], F32, tag="g")
                nc.scalar.activation(g_sb, ps_g, AF.Sigmoid, bias=bias_ap(COL_G + jc))
                c_sb = tmp.tile([P, B], F32, tag="c")
                nc.scalar.activation(c_sb, ps_c, AF.Silu, bias=bias_ap(COL_C + jc))
                h_sb = tmp.tile([P, B], F32, tag="h")
                nc.vector.tensor_mul(h_sb, onemf, c_sb)
                nc.vector.tensor_mul(gh_sb[:, jc], g_sb, h_sb)

            # The head runs ko-major across 7 open PSUM groups (jc 0-1 all
            # gates + jc 2's f gate; warmup bank + 7 = all 8 banks): each
            # arriving x chunk unlocks ~1.58us of matmuls, above its
            # arrival cadence, so the PE rides the x-transfer front
            # gap-free (recurring idle gaps re-throttle the HAM clock).
            GATES = (("f", wf), ("c", wc), ("g", wg))
            HEAD = [(0, "f"), (0, "c"), (0, "g"),
                    (1, "f"), (1, "c"), (1, "g"), (2, "f")]
            SRC = dict(GATES)
            stages, hw, hp = {}, {}, {}
            for jc, nm in HEAD:
                src = SRC[nm]
                if jc == 0 and nm == "f":
                    stages[jc, nm] = wf0_8
                else:
                    st = w8pool.tile(
                        [P, JC_D, P], F8, tag="w8", name=f"hs_{jc}{nm}"
                    )
                    nc.sync.dma_start(out=st[:], in_=src[jc])
                    stages[jc, nm] = st
                hw[jc, nm] = wpool.tile(
                    [P, JC_D, P], F32R, tag="w512", name=f"hw_{jc}{nm}"
                )
                hp[jc, nm] = psum.tile(
                    [P, B], F32, tag="ps", name=f"hp_{jc}{nm}"
                )
            # half-slab expands, all first halves before second halves,
            # so every slab's ko<8 columns are ready early
            HK = JC_D // 2
            for half in (slice(0, HK), slice(HK, JC_D)):
                for jc, nm in HEAD:
                    nc.vector.tensor_copy(
                        hw[jc, nm][:, half], stages[jc, nm][:, half]
                    )
            for ko in range(JC_D):
                for jc, nm in HEAD:
                    nc.tensor.matmul(
                        hp[jc, nm], hw[jc, nm][:, ko], x_sb[:, ko],
                        start=(ko == 0), stop=(ko == JC_D - 1),
                    )
            for jc in (0, 1):
                gate_epilogue(jc, hp[jc, "f"], hp[jc, "c"], hp[jc, "g"])

            for jc in range(2, JC_D):
                if jc == 2:
                    ps_f = hp[2, "f"]
                else:
                    wf_sb = expand(wf, jc)
                    ps_f = psum.tile([P, B], F32, tag="ps")
                    mm(ps_f, wf_sb, x_sb, JC_D)

                wc_sb = expand(wc, jc)
                ps_c = psum.tile([P, B], F32, tag="ps")
                mm(ps_c, wc_sb, x_sb, JC_D)

                wg_sb = expand(wg, jc)
                ps_g = psum.tile([P, B], F32, tag="ps")
                mm(ps_g, wg_sb, x_sb, JC_D)

                gate_epilogue(jc, ps_f, ps_c, ps_g)

        # ---- phase 3/4 shared defs (needed for prefetch from phase 2) ----
        # pass list: (weight set, jq), ACT-table-friendly order
        passes = []
        for jq in range(JQ3):
            pair = [("u", jq), ("g", jq)]
            if jq % 2:
                pair.reverse()
            passes += pair
        W3SRC = {"u": wuS, "g": wg2S}

        def issue_p3A(pi):
            """First slab piece (M1..M4, consumed from pass start)."""
            kind, jq = passes[pi]
            src = W3SRC[kind]
            t = w3pool.tile([P, 4, KK3, P], F8, tag="w3a")
            for s in range(4):
                pp = slice(s * (P // 4), (s + 1) * (P // 4))
                nc.sync.dma_start(out=t[pp], in_=src[jq, pp, 0:4])
            return t

        def issue_p3B(pi):
            """Second slab piece (M5..M7, consumed ~mid-pass)."""
            kind, jq = passes[pi]
            src = W3SRC[kind]
            t = w3pool.tile([P, 3, KK3, P], F8, tag="w3b")
            for s in range(4):
                pp = slice(s * (P // 4), (s + 1) * (P // 4))
                nc.sync.dma_start(out=t[pp], in_=src[jq, pp, 4:7])
            return t

        def issue_p4_slab(jq, m):
            # 8-way split (halves the in-flight latency vs the 3.4us/M
            # consumption rate); half issued from the Activation engine
            # (also hwdge-capable, idle in phase 4) to keep SP under ~60%
            t = w4pool.tile([P, KK4, P], F8, tag="w4")
            for s in range(4):
                pp = slice(s * (P // 4), (s + 1) * (P // 4))
                eng = nc.sync if s % 2 == 0 else nc.scalar
                eng.dma_start(out=t[pp], in_=wo2S[m, jq, pp])
            return t

        slabA = {}
        slabB = {}
        w4_prime = []

        # ---- phase 2: o = out_proj(gh) + b -> fp16; build phase-3 combos ----
        def om_combos(kk):
            """o chunks kk (k-half 1) and kk+8 (k-half 2) are both ready:
            emit the 5 fp16 Strassen combos for phase 3 at column kk."""
            o11 = o_sb[:, kk, 0:BH]
            o12 = o_sb[:, kk, BH:B]
            o21 = o_sb[:, kk + KK3, 0:BH]
            o22 = o_sb[:, kk + KK3, BH:B]
            nc.vector.scalar_tensor_tensor(OM[0][:, kk], o11, 1.0, o22, ALU.mult, ALU.add)
            nc.vector.scalar_tensor_tensor(OM[2][:, kk], o12, 1.0, o22, ALU.mult, ALU.subtract)
            nc.vector.scalar_tensor_tensor(OM[3][:, kk], o21, 1.0, o11, ALU.mult, ALU.subtract)
            nc.vector.scalar_tensor_tensor(OM[5][:, kk], o11, 1.0, o12, ALU.mult, ALU.add)
            nc.vector.scalar_tensor_tensor(OM[6][:, kk], o21, 1.0, o22, ALU.mult, ALU.add)

        # w3pool opens at phase-2 start (x_pool/tmp1 are gone, so it fits)
        # and passes 0/1 are primed HERE: SP executes dma_starts in program
        # order and stalls on w8pool WAR waits, so it only reaches these
        # after phase 1 — they land during phase 2, not at the boundary.
        w3pool = es_right.enter_context(tc.tile_pool(name="w3pool", bufs=3, side="right"))
        slabA[0] = issue_p3A(0)
        slabB[0] = issue_p3B(0)
        slabA[1] = issue_p3A(1)
        slabB[1] = issue_p3B(1)

        for jc in range(JC_D):
            wo_sb = expand(wo, jc)
            ps_o = psum.tile([P, B], F32, tag="ps")
            mm(ps_o, wo_sb, gh_sb, JC_D)
            nc.vector.tensor_scalar_add(o_sb[:, jc], ps_o, bias_ap(COL_O + jc))
            if jc >= KK3:
                om_combos(jc - KK3)

        es_p12.close()  # frees wpool, w8pool, gh (also x_pool/tmp1 already closed)
        es_psumA.close()

        # ---- phases 3-4: Strassen level-1 machinery ----
        # (pools open only now: SBUF space is reserved at pool-open, and these
        # only fit once the phase-1/2 pools are gone)
        es_psumB = ExitStack()
        psumS = es_psumB.enter_context(tc.tile_pool(name="psumS", bufs=8, space="PSUM"))
        pb_pool = es_right.enter_context(tc.tile_pool(name="pb_pool", bufs=1, side="right"))
        # phase-4 moving operands (Strassen B-side of gu): 7 fp16 tensors
        # (indices 1 and 4 are the raw B11/B22 quadrants, written directly)
        PB = [pb_pool.tile([P, KK4, BH], F16, name=f"pb{m}") for m in range(7)]
        w4pool = es_right.enter_context(tc.tile_pool(name="w4pool", bufs=4, side="right"))
        s_tmp = es_right.enter_context(tc.tile_pool(name="s_tmp", bufs=12, side="right"))

        def strassen_group(kk_n, slab, mover, combine):
            """Emit the 7 quadrant products and C-assembly for one j-group.
            slabs[m]: SBUF fp8 stationary [P, kk_n, P]; mover(m, kk): fp16
            moving [P, BH] AP; combine(quad, in0_sbuf, in1_psum, op1, eng)
            emits the final combining op for C_quad. M1/M2/M3 get SBUF
            copies (a DVE op may read at most one PSUM source); M6/M7
            accumulate onto the M1/M5' banks. Assembly ops are split
            between DVE ("v") and the otherwise-idle GpSimd ("g") so
            neither queue backlogs the PSUM-bank recycling the PE needs."""
            def run_m(m, ps=None):
                start = ps is None
                if start:
                    ps = psumS.tile([P, BH], F32, tag="psS")
                for kk in range(kk_n):
                    nc.tensor.matmul(
                        ps, slab(m, kk), mover(m, kk),
                        start=(start and kk == 0), stop=(kk == kk_n - 1),
                        skip_group_check=not start,
                    )
                return ps

            def to_sbuf(ps, scalar=False):
                t = s_tmp.tile([P, BH], F32, tag="t")
                if scalar:
                    # ScalarE copy: 'copy' is in every ACT table (no reload),
                    # and this keeps the PSUM-freeing path off the DVE queue
                    nc.scalar.activation(t, ps, AF.Copy)
                else:
                    nc.vector.tensor_copy(t, ps)
                return t

            mA = run_m(0)           # bank A = M1
            m1s = to_sbuf(mA, scalar=True)
            mB = run_m(1)           # bank B = M2
            m2s = to_sbuf(mB, scalar=True)
            mC = run_m(3)           # bank C = M4
            c11a = s_tmp.tile([P, BH], F32, tag="t")
            nc.vector.scalar_tensor_tensor(c11a, m1s, 1.0, mC, ALU.mult, ALU.add)
            combine(21, m2s, mC, ALU.add, "v")       # C21 = M2 + M4
            mD = run_m(2)           # bank D = M3
            m3s = to_sbuf(mD)
            mE = run_m(4)           # bank E = M5' = -M5
            combine(12, m3s, mE, ALU.subtract, "v")  # C12 = M3 - M5'
            t22 = s_tmp.tile([P, BH], F32, tag="t")
            nc.vector.scalar_tensor_tensor(t22, m3s, 1.0, m2s, ALU.mult, ALU.subtract)
            run_m(5, ps=mA)         # bank A = M1 + M6
            combine(22, t22, mA, ALU.add, "v")       # C22 = (M3-M2)+(M1+M6)
            run_m(6, ps=mE)         # bank E = M5' + M7
            combine(11, c11a, mE, ALU.add, "v")      # C11 = (M1+M4)+(M5'+M7)

        # ---- phase 3: BitGLU u/g2 via Strassen; gu -> phase-4 operands ----
        # movers: combos + raw o-quadrant views (B11 = o[kh1, bh1],
        # B22 = o[kh2, bh2])
        def mover3(m, kk):
            if m == 1:
                return o_sb[:, kk, 0:BH]
            if m == 4:
                return o_sb[:, KK3 + kk, BH:B]
            return OM[m][:, kk]

        with (
            tc.tile_pool(name="stash", bufs=5, side="right") as stash_pool,
            tc.tile_pool(name="p3b", bufs=4, side="right") as p3b,
        ):
            stash = {}
            bt = {}
            for pi, (kind, jq) in enumerate(passes):
                # prefetch: late piece of pass pi+1 first (needed sooner —
                # SP and the queues are FIFO), then early piece of pi+2
                if pi + 1 < len(passes) and pi + 1 not in slabB:
                    slabB[pi + 1] = issue_p3B(pi + 1)
                if pi + 2 < len(passes) and pi + 2 not in slabA:
                    slabA[pi + 2] = issue_p3A(pi + 2)
                tA = slabA.pop(pi)
                tB = slabB.pop(pi)
                first = (pi % 2) == 0
                func = AF.Silu if kind == "u" else AF.Sigmoid
                colb = COL_U if kind == "u" else COL_G2

                def combine(quad, a, b_, op1, eng, _jq=jq, _first=first, _func=func, _colb=colb):
                    c = s_tmp.tile([P, BH], F32, tag="t")
                    e = nc.vector if eng == "v" else nc.gpsimd
                    e.scalar_tensor_tensor(c, a, 1.0, b_, ALU.mult, op1)
                    jout = _jq if quad in (11, 12) else _jq + JQ3
                    if _first:
                        dst = stash_pool.tile([P, BH], F32, tag="s1")
                        nc.scalar.activation(dst, c, _func, bias=bias_ap(_colb + jout))
                        stash[quad] = dst
                    else:
                        act = s_tmp.tile([P, BH], F32, tag="t")
                        nc.scalar.activation(act, c, _func, bias=bias_ap(_colb + jout))
                        # gu quadrant = (act/16) * stashed other factor
                        if quad == 11:
                            dst = PB[1][:, _jq]
                        elif quad == 22:
                            dst = PB[4][:, _jq]
                        else:
                            dst = p3b.tile([P, BH], F16, tag="bt", name=f"bt{quad}")
                            bt[quad] = dst
                        nc.vector.scalar_tensor_tensor(
                            dst, act, 1.0 / GU_SCALE, stash[quad], ALU.mult, ALU.mult
                        )
                        if quad == 11:
                            # last quad of the pair: emit phase-4 combos at
                            # kk=jq (GpSimd: keep them off the DVE queue)
                            b11 = PB[1][:, _jq]
                            b22 = PB[4][:, _jq]
                            b12 = bt[12]
                            b21 = bt[21]
                            nc.vector.scalar_tensor_tensor(PB[0][:, _jq], b11, 1.0, b22, ALU.mult, ALU.add)
                            nc.vector.scalar_tensor_tensor(PB[2][:, _jq], b12, 1.0, b22, ALU.mult, ALU.subtract)
                            nc.vector.scalar_tensor_tensor(PB[3][:, _jq], b21, 1.0, b11, ALU.mult, ALU.subtract)
                            nc.vector.scalar_tensor_tensor(PB[5][:, _jq], b11, 1.0, b12, ALU.mult, ALU.add)
                            nc.vector.scalar_tensor_tensor(PB[6][:, _jq], b21, 1.0, b22, ALU.mult, ALU.add)

                strassen_group(
                    KK3,
                    lambda m, kk, _a=tA, _b=tB: (
                        _a[:, m, kk] if m < 4 else _b[:, m - 4, kk]
                    ),
                    mover3, combine,
                )
                if pi == len(passes) - 2:
                    # prime phase-4 slab prefetch (first two M's of group 0)
                    w4_prime.append(issue_p4_slab(0, 0))
                    w4_prime.append(issue_p4_slab(0, 1))

        with tc.tile_pool(name="outp", bufs=4, side="right") as outp:
            pending = {}
            for jq in range(JQ4):
                for m in range(7):
                    if jq == 0 and m < 2:
                        pending[(jq, m)] = w4_prime[m]
                    g2, m2_ = (jq, m + 2) if m + 2 < 7 else (jq + 1, m + 2 - 7)
                    if g2 < JQ4 and (g2, m2_) not in pending:
                        pending[(g2, m2_)] = issue_p4_slab(g2, m2_)
                slabs = [pending.pop((jq, m)) for m in range(7)]

                def combine4(quad, a, b_, op1, eng, _jq=jq):
                    jout = _jq if quad in (11, 12) else _jq + JQ4
                    boff = 0 if quad in (11, 21) else BH
                    y = outp.tile([P, BH], F32, tag="y")
                    e = nc.vector if eng == "v" else nc.gpsimd
                    e.scalar_tensor_tensor(
                        y, a, bias_ap(COL_Y + jout), b_, ALU.add, op1
                    )
                    ns = 4 if _jq == JQ4 - 1 else 2
                    for s in range(ns):
                        pp = slice(s * (P // ns), (s + 1) * (P // ns))
                        nc.sync.dma_start(
                            out=out[jout, pp, boff : boff + BH], in_=y[pp]
                        )

                strassen_group(
                    KK4, lambda m, kk, _s=slabs: _s[m][:, kk],
                    lambda m, kk: PB[m][:, kk], combine4,
                )

        es_right.close()
        es_psumB.close()
        es_all.close()

    _split_excess_waits(nc)
    return nc


def prep_in_maps(inputs):
    """Build the 8 per-core input maps from the full-size inputs."""
    import ml_dtypes

    E4 = ml_dtypes.float8_e4m3
    x = np.asarray(inputs["x"], np.float32)

    shared = {
        "wf": _pack_weight(inputs["f_gate_w"], dtype=E4),
        "wc": _pack_weight(inputs["c_proj_w"], dtype=E4),
        "wg": _pack_weight(inputs["g_gate_w"], dtype=E4),
        "wo": _pack_weight(inputs["out_proj_w"], dtype=E4),
        "wuS": _pack_strassen(inputs["proj_u_w"], E4, conjoined=True),
        "wg2S": _pack_strassen(inputs["proj_g_w"], E4, conjoined=True),
        # x16: undoes the gu/16 fp16 storage scale so no epilogue multiply
        "wo2S": _pack_strassen(inputs["proj_out_w"], E4, scale=GU_SCALE),
    }
    bias = np.zeros((P, N_BIAS_COLS), np.float32)
    bias[:, COL_NF:COL_NF + JC_D] = _pack_bias_col(-np.asarray(inputs["f_gate_b"]))
    bias[:, COL_C:COL_C + JC_D] = _pack_bias_col(inputs["c_proj_b"])
    bias[:, COL_G:COL_G + JC_D] = _pack_bias_col(inputs["g_gate_b"])
    bias[:, COL_O:COL_O + JC_D] = _pack_bias_col(inputs["out_proj_b"])
    bias[:, COL_U:COL_U + JC_H] = _pack_bias_col(inputs["proj_u_b"])
    bias[:, COL_G2:COL_G2 + JC_H] = _pack_bias_col(inputs["proj_g_b"])
    bias[:, COL_Y:COL_Y + JC_D] = _pack_bias_col(inputs["proj_out_b"])
    shared["biases"] = bias

    in_maps = []
    for core in range(NCORES):
        m = dict(shared)
        m["xT"] = _pack_x(x[core * B : (core + 1) * B])
        in_maps.append(m)
    return in_maps


def gather_output(results):
    """results[i]['out'] is [JC_D, P, B]; assemble full [BATCH, DIM] f32."""
    parts = []
    for core in range(NCORES):
        y = np.asarray(results[core]["out"], np.float32)  # [jc, p, b]
        parts.append(y.reshape(DIM, B).T)  # [b, j]
    return np.ascontiguousarray(np.concatenate(parts, axis=0))


_NC_CACHE = []


def run(inputs, trace=False, **kw):
    from concourse.bass_utils import run_bass_kernel_spmd

    if not _NC_CACHE:
        _NC_CACHE.append(_build_nc())
    nc = _NC_CACHE[0]
    in_maps = prep_in_maps(inputs)
    res = run_bass_kernel_spmd(nc, in_maps, core_ids=list(range(NCORES)), trace=trace, **kw)
    return res


def kernel(**inputs):
    res = run(inputs, trace=False)
    return gather_output(res.results)
